# revision 1
# baseline (speedup 1.0000x reference)
"""DiagonalLinear kernel for 8x TRN2 NeuronCores (Bass/Tile).

Math: out[b, i] = sum_j x[b, j] * (weight * mask)[i, j] where
mask[i, lag*N_VARS + i] = 1. So the dense matmul collapses to

    out[b, i] = sum_{lag<P} x[b, lag*N_VARS + i] * wd[i, lag]
    wd[i, lag] = weight[i, lag*N_VARS + i]

i.e. an elementwise multiply-accumulate over P=8 lags — memory-bound on
streaming x (256 MB fp32) once, not a matmul.

Sharding: each of the 8 cores owns a contiguous slice of NV=256 variables
(fully independent given the diagonal mask). Per-core device layout puts
variables on SBUF partitions and batch on the free dim, so the per-lag
multiply needs only a per-partition scalar: lag 0 is a tensor_scalar_mul
(2x fp32 mode) and lags 1..7 are fused scalar_tensor_tensor
(acc = x*wd + acc), all on VectorE (~68 us busy). DMA is the bottleneck:
~36 MB per core at the ~360 GB/s HBM-per-core limit (~105 us). Loads are
issued per lag-pair (1 MB each) so compute streams behind the loads;
the last chunk loads per-lag and splits its final accumulate+store into
b-halves so the kernel tail drains with the last loads. Cost-model
(TimelineSim) predicted time: 110,352 ns/core (DMA busy 104.9 us).

Host side: extract the weight diagonal (pure gather), transpose x so each
core's shard is (P*NV, BATCH) contiguous, gather per-core outputs (NV,
BATCH) and transpose back to (BATCH, N_VARS).
"""

import os

import numpy as np

import concourse.bass as bass
import concourse.mybir as mybir
from concourse.bass_utils import run_bass_kernel_spmd
from concourse.tile import TileContext

N_VARS = 2048
P = 8
BATCH = 4096
N_CORES = 8
NV = N_VARS // N_CORES  # 256 variables per core
VT = NV // 128  # 2 partition tiles per core
BB = 1024  # batch tile width (free dim)
NB = BATCH // BB
LAG_GROUP = 2  # lags per load DMA (2 -> 1 MB transfers)

_nc_cache = None
LAST_EXEC_TIME_NS = None


def _split_multi_waits(nc):
    """Walrus in this toolchain accepts at most one sync-wait per
    instruction; hoist extras onto same-engine NoOps placed just before.
    Order-preserving and conservative: the engine stalls at the NoOp on the
    same condition it would have waited on at the instruction itself."""
    for fn in nc.m.functions:
        for blk in fn.blocks:
            out = []
            for ins in blk.instructions:
                si = ins.sync_info
                if si is not None and si.on_wait is not None and len(si.on_wait) > 1:
                    waits = list(si.on_wait)
                    for k, w in enumerate(waits[:-1]):
                        out.append(
                            mybir.InstNoOp(
                                name=f"{ins.name}_hw{k}",
                                engine=ins.engine,
                                ins=[],
                                outs=[],
                                sync_info=mybir.SyncInfo(on_wait=[w], on_update=[]),
                            )
                        )
                    ins.sync_info = mybir.SyncInfo(
                        on_wait=[waits[-1]], on_update=si.on_update
                    )
                out.append(ins)
            blk.instructions[:] = out


def _build_nc():
    nc = bass.Bass()
    xt = nc.dram_tensor("xt", [P * NV, BATCH], mybir.dt.float32, kind="ExternalInput")
    wds = nc.dram_tensor("wds", [128, VT * P], mybir.dt.float32, kind="ExternalInput")
    out = nc.dram_tensor("out_t", [NV, BATCH], mybir.dt.float32, kind="ExternalOutput")
    # view rows as (lag, v): row = lag*NV + v  ->  [v, lag, b]
    xt_v = xt.rearrange("(l v) b -> v l b", l=P)

    n_chunks = VT * NB
    with TileContext(nc) as tc:
        with (
            tc.tile_pool(name="w", bufs=1) as wpool,
            tc.tile_pool(name="x", bufs=3) as xpool,
            tc.tile_pool(name="acc", bufs=2) as apool,
        ):
            wtile = wpool.tile([128, VT * P], mybir.dt.float32)
            # ACT ring: keeps the SP ring free so the first x load issues
            # immediately
            nc.scalar.dma_start(out=wtile[:, :], in_=wds[:, :])
            for ci, (vt, bb) in enumerate(
                (vt, bb) for vt in range(VT) for bb in range(NB)
            ):
                # the last chunk loads per-lag so its accumulation chain
                # streams with the loads instead of waiting for all 8 lags
                lg = 1 if ci == n_chunks - 1 else LAG_GROUP
                t = xpool.tile([128, P, BB], mybir.dt.float32, tag="xload")
                for l0 in range(0, P, lg):
                    nc.sync.dma_start(
                        out=t[:, l0 : l0 + lg, :],
                        in_=xt_v[
                            vt * 128 : (vt + 1) * 128,
                            l0 : l0 + lg,
                            bb * BB : (bb + 1) * BB,
                        ],
                    )
                acc = apool.tile([128, BB], mybir.dt.float32, tag="acc")
                # acc = wd[:, lag0] * x_lag0  (per-partition scalar, 2x fp32)
                nc.vector.tensor_scalar_mul(
                    out=acc[:, :],
                    in0=t[:, 0, :],
                    scalar1=wtile[:, vt * P : vt * P + 1],
                )
                for lag in range(1, P - 1):
                    # acc = (x_lag * wd[:, lag]) + acc  (fused on VectorE)
                    nc.vector.scalar_tensor_tensor(
                        out=acc[:, :],
                        in0=t[:, lag, :],
                        scalar=wtile[:, vt * P + lag : vt * P + lag + 1],
                        in1=acc[:, :],
                        op0=mybir.AluOpType.mult,
                        op1=mybir.AluOpType.add,
                    )
                # final lag: on the last chunk, split the closing STT and
                # store into b-halves so the first half's store overlaps the
                # second half's accumulate — shortens the kernel tail
                lag = P - 1
                wl = wtile[:, vt * P + lag : vt * P + lag + 1]
                nsp = 2 if ci == n_chunks - 1 else 1
                S = BB // nsp
                for s in range(nsp):
                    nc.vector.scalar_tensor_tensor(
                        out=acc[:, s * S : (s + 1) * S],
                        in0=t[:, lag, s * S : (s + 1) * S],
                        scalar=wl,
                        in1=acc[:, s * S : (s + 1) * S],
                        op0=mybir.AluOpType.mult,
                        op1=mybir.AluOpType.add,
                    )
                    # store on the ACT HWDGE ring so a hoisted store-wait
                    # cannot stall load issue on the SP ring
                    nc.scalar.dma_start(
                        out=out[
                            vt * 128 : (vt + 1) * 128,
                            bb * BB + s * S : bb * BB + (s + 1) * S,
                        ],
                        in_=acc[:, s * S : (s + 1) * S],
                    )
    _split_multi_waits(nc)
    return nc


def _get_nc():
    global _nc_cache
    if _nc_cache is None:
        _nc_cache = _build_nc()
    return _nc_cache


def kernel(**inputs) -> np.ndarray:
    global LAST_EXEC_TIME_NS
    x = np.asarray(inputs["x"], dtype=np.float32)
    weight = np.asarray(inputs["weight"], dtype=np.float32)
    assert x.shape == (BATCH, N_VARS * P)
    assert weight.shape == (N_VARS, N_VARS * P)

    # wd[i, lag] = weight[i, lag*N_VARS + i]  (diagonal gather, no arithmetic)
    wd = np.einsum("ili->il", weight.reshape(N_VARS, P, N_VARS))

    # xT[j, b] = x[b, j]; j = lag*N_VARS + core*NV + v
    xT = np.ascontiguousarray(x.T)
    xTr = xT.reshape(P, N_CORES, NV, BATCH)

    in_maps = []
    for c in range(N_CORES):
        xt_c = np.ascontiguousarray(xTr[:, c]).reshape(P * NV, BATCH)
        wd_c = wd[c * NV : (c + 1) * NV]  # (NV, P)
        wds_c = np.ascontiguousarray(
            wd_c.reshape(VT, 128, P).transpose(1, 0, 2).reshape(128, VT * P)
        )
        in_maps.append({"xt": xt_c, "wds": wds_c})

    nc = _get_nc()
    trace = bool(int(os.environ.get("KERNEL_TRACE", "0")))

    def _run(tr):
        return run_bass_kernel_spmd(
            nc, in_maps, core_ids=list(range(N_CORES)), trace=tr
        )

    try:
        res = _run(trace)
    except ModuleNotFoundError:
        # axon containers without the NTFF profile hook can't trace
        # (BASS_TRACE env still forces trace inside run_bass_kernel_spmd)
        os.environ["BASS_NEVER_TRACE"] = "1"
        res = _run(False)
    except Exception:
        # transient device errors (e.g. NRT_EXEC_UNIT_UNRECOVERABLE after a
        # wedged execution unit) clear on re-run; retry once before failing
        import time as _time

        _time.sleep(2.0)
        res = _run(trace)
    LAST_EXEC_TIME_NS = res.exec_time_ns

    out_full = np.empty((BATCH, N_VARS), dtype=np.float32)
    for c in range(N_CORES):
        out_full[:, c * NV : (c + 1) * NV] = np.asarray(res.results[c]["out_t"]).T
    return out_full



# revision 3
# speedup vs baseline: 1.8416x; 1.8416x over previous
"""DiagonalLinear kernel for 8x TRN2 NeuronCores (Bass/Tile).

Math: out[b, i] = sum_j x[b, j] * (weight * mask)[i, j] where
mask[i, lag*N_VARS + i] = 1. So the dense matmul collapses to

    out[b, i] = sum_{lag<P} x[b, lag*N_VARS + i] * wd[i, lag]
    wd[i, lag] = weight[i, lag*N_VARS + i]

i.e. an elementwise multiply-accumulate over P=8 lags — memory-bound on
streaming x once, not a matmul.

Sharding: each of the 8 cores owns a contiguous slice of NV=256 variables
(fully independent given the diagonal mask). The rel-err budget (2e-2) is
~50x wider than bf16 quantization error (~3e-3 measured end to end), so x
is staged to HBM in bf16 — halving the dominant DMA traffic vs fp32 — and
the output is returned in bf16 and upcast on the host. Weights stay fp32
on the scalar path (they're tiny).

Per-core device pipeline (vars on partitions, batch on the free dim):
  - x streams in as 16 (vt, lag) tiles of [128, 4096] bf16 (1 MiB each)
    on the SP HWDGE ring.
  - TensorE multiplies each lag tile by a [128,128] *diagonal* stationary
    matrix diag(wd[:, lag]) (built on device: bf16 identity DMA'd once,
    scaled per-partition by wd on VectorE), accumulating lags 0..6 into
    8 PSUM banks (one per 512-wide batch chunk). LDWEIGHTS/stationary
    reload is free on the modeled timeline and the MMs (~27-48 us PE)
    hide under the DMA stream.
  - The PSUM->SBUF eviction fuses lag 7: one scalar_tensor_tensor per
    bank on VectorE computes bf16(x_lag7 * wd7 + psum) straight into the
    SBUF output tile, which stores per-bank on the ACT HWDGE ring.
  - The final (vt=1, lag=7) x tile loads per-bank so the closing
    STT+store chain drains with the last 128 KiB chunks instead of
    waiting for the full tile.

DMA totals per core: 16 MiB x in + 2 MiB out + ~40 KiB weights/identity
at the ~360 GB/s modeled DMA rate -> ~52 us of DMA busy, which bounds the
kernel; compute engines (PE ~27-48 us, DVE ~13 us, ACT issue ~11 us) all
hide behind it.

Host side: extract the weight diagonal (pure gather), cast x to bf16 and
transpose so each core's shard is (P*NV, BATCH) contiguous, gather
per-core bf16 outputs (NV, BATCH), transpose back and upcast to fp32.
"""

import os

import ml_dtypes
import numpy as np

import concourse.bass as bass
import concourse.mybir as mybir
from concourse.bass_utils import run_bass_kernel_spmd
from concourse.tile import TileContext

N_VARS = 2048
P = 8
BATCH = 4096
N_CORES = 8
NV = N_VARS // N_CORES  # 256 variables per core
VT = NV // 128  # 2 partition tiles per core
BB = 512  # batch chunk per PSUM bank (512 fp32 = one full bank)
NB = BATCH // BB  # 8 banks

BF16 = ml_dtypes.bfloat16

_nc_cache = None
LAST_EXEC_TIME_NS = None


def _split_multi_waits(nc):
    """Walrus in this toolchain accepts at most one sync-wait per
    instruction; hoist extras onto same-engine NoOps placed just before.
    Order-preserving and conservative: the engine stalls at the NoOp on the
    same condition it would have waited on at the instruction itself."""
    for fn in nc.m.functions:
        for blk in fn.blocks:
            out = []
            for ins in blk.instructions:
                si = ins.sync_info
                if si is not None and si.on_wait is not None and len(si.on_wait) > 1:
                    waits = list(si.on_wait)
                    for k, w in enumerate(waits[:-1]):
                        out.append(
                            mybir.InstNoOp(
                                name=f"{ins.name}_hw{k}",
                                engine=ins.engine,
                                ins=[],
                                outs=[],
                                sync_info=mybir.SyncInfo(on_wait=[w], on_update=[]),
                            )
                        )
                    ins.sync_info = mybir.SyncInfo(
                        on_wait=[waits[-1]], on_update=si.on_update
                    )
                out.append(ins)
            blk.instructions[:] = out


def _build_nc():
    nc = bass.Bass()
    xt = nc.dram_tensor("xt", [P * NV, BATCH], mybir.dt.bfloat16, kind="ExternalInput")
    wds = nc.dram_tensor("wds", [128, VT * P], mybir.dt.float32, kind="ExternalInput")
    ident = nc.dram_tensor("ident", [128, 128], mybir.dt.bfloat16, kind="ExternalInput")
    out = nc.dram_tensor("out_t", [NV, BATCH], mybir.dt.bfloat16, kind="ExternalOutput")
    # view rows as (lag, v): row = lag*NV + v  ->  [v, lag, b]
    xt_v = xt.rearrange("(l v) b -> v l b", l=P)

    with TileContext(nc) as tc:
        with (
            tc.tile_pool(name="w", bufs=1) as wpool,
            tc.tile_pool(name="x", bufs=VT * P) as xpool,
            tc.tile_pool(name="acc", bufs=2) as apool,
            tc.tile_pool(name="ps", bufs=NB, space=bass.MemorySpace.PSUM) as ppool,
        ):
            wtile = wpool.tile([128, VT * P], mybir.dt.float32)
            itile = wpool.tile([128, 128], mybir.dt.bfloat16)
            dtile = wpool.tile([128, VT, P, 128], mybir.dt.bfloat16)
            # small weight/identity loads on the ACT ring so the SP ring is
            # free for the first x load
            nc.scalar.dma_start(out=wtile[:, :], in_=wds[:, :])
            nc.scalar.dma_start(out=itile[:, :], in_=ident[:, :])
            # stationaries: diag(wd[:, vt, lag]) = identity * per-partition wd
            for vt in range(VT):
                for lag in range(P):
                    nc.vector.tensor_scalar_mul(
                        out=dtile[:, vt, lag, :],
                        in0=itile[:, :],
                        scalar1=wtile[:, vt * P + lag : vt * P + lag + 1],
                    )

            # issue every x load up front (SP ring, program order = stream
            # order); all 16 tiles stay resident so loads never wait
            xtiles = {}
            for vt in range(VT):
                for lag in range(P):
                    t = xpool.tile([128, BATCH], mybir.dt.bfloat16, tag="x")
                    if vt == VT - 1 and lag == P - 1:
                        # last tile feeds the closing per-bank STT+store
                        # chain: load per-bank so the tail drains with the
                        # final chunks
                        for bb in range(NB):
                            nc.sync.dma_start(
                                out=t[:, bb * BB : (bb + 1) * BB],
                                in_=xt_v[
                                    vt * 128 : (vt + 1) * 128,
                                    lag,
                                    bb * BB : (bb + 1) * BB,
                                ],
                            )
                    else:
                        nc.sync.dma_start(
                            out=t[:, :],
                            in_=xt_v[vt * 128 : (vt + 1) * 128, lag, :],
                        )
                    xtiles[(vt, lag)] = t

            for vt in range(VT):
                banks = [
                    ppool.tile(
                        [128, BB], mybir.dt.float32, tag="psum", name=f"ps_{vt}_{bb}"
                    )
                    for bb in range(NB)
                ]
                # lags 0..6 accumulate in PSUM; lag ordering streams behind
                # the per-lag loads
                for lag in range(P - 1):
                    d = dtile[:, vt, lag, :]
                    xl = xtiles[(vt, lag)]
                    for bb in range(NB):
                        nc.tensor.matmul(
                            out=banks[bb][:, :],
                            lhsT=d,
                            rhs=xl[:, bb * BB : (bb + 1) * BB],
                            start=(lag == 0),
                            stop=(lag == P - 2),
                        )
                # eviction fuses lag 7: bf16 out = x_lag7 * wd7 + psum
                acc = apool.tile([128, BATCH], mybir.dt.bfloat16, tag="acc")
                xl = xtiles[(vt, P - 1)]
                wl = wtile[:, vt * P + P - 1 : vt * P + P]
                for bb in range(NB):
                    nc.vector.scalar_tensor_tensor(
                        out=acc[:, bb * BB : (bb + 1) * BB],
                        in0=xl[:, bb * BB : (bb + 1) * BB],
                        scalar=wl,
                        in1=banks[bb][:, :],
                        op0=mybir.AluOpType.mult,
                        op1=mybir.AluOpType.add,
                    )
                    # store on the ACT HWDGE ring so store-waits cannot
                    # stall load issue on the SP ring
                    nc.scalar.dma_start(
                        out=out[
                            vt * 128 : (vt + 1) * 128,
                            bb * BB : (bb + 1) * BB,
                        ],
                        in_=acc[:, bb * BB : (bb + 1) * BB],
                    )
    _split_multi_waits(nc)
    return nc


def _get_nc():
    global _nc_cache
    if _nc_cache is None:
        _nc_cache = _build_nc()
    return _nc_cache


def kernel(**inputs) -> np.ndarray:
    global LAST_EXEC_TIME_NS
    x = np.asarray(inputs["x"], dtype=np.float32)
    weight = np.asarray(inputs["weight"], dtype=np.float32)
    assert x.shape == (BATCH, N_VARS * P)
    assert weight.shape == (N_VARS, N_VARS * P)

    # wd[i, lag] = weight[i, lag*N_VARS + i]  (diagonal gather, no arithmetic)
    wd = np.einsum("ili->il", weight.reshape(N_VARS, P, N_VARS))

    # bf16 staging: cast once, then transpose; j = lag*N_VARS + core*NV + v
    xb = x.astype(BF16)
    xTr = xb.T.reshape(P, N_CORES, NV, BATCH)  # reshape of a view -> one copy

    ident = np.eye(128, dtype=BF16)
    in_maps = []
    for c in range(N_CORES):
        xt_c = np.ascontiguousarray(xTr[:, c]).reshape(P * NV, BATCH)
        wd_c = wd[c * NV : (c + 1) * NV]  # (NV, P) fp32
        wds_c = np.ascontiguousarray(
            wd_c.reshape(VT, 128, P).transpose(1, 0, 2).reshape(128, VT * P)
        )
        in_maps.append({"xt": xt_c, "wds": wds_c, "ident": ident})

    nc = _get_nc()
    trace = bool(int(os.environ.get("KERNEL_TRACE", "0")))

    def _run(tr):
        return run_bass_kernel_spmd(
            nc, in_maps, core_ids=list(range(N_CORES)), trace=tr
        )

    try:
        res = _run(trace)
    except ModuleNotFoundError:
        # axon containers without the NTFF profile hook can't trace
        # (BASS_TRACE env still forces trace inside run_bass_kernel_spmd)
        os.environ["BASS_NEVER_TRACE"] = "1"
        res = _run(False)
    except Exception:
        # transient device errors (e.g. NRT_EXEC_UNIT_UNRECOVERABLE after a
        # wedged execution unit) clear on re-run; retry once before failing
        import time as _time

        _time.sleep(2.0)
        res = _run(trace)
    LAST_EXEC_TIME_NS = res.exec_time_ns

    out_full = np.empty((BATCH, N_VARS), dtype=np.float32)
    for c in range(N_CORES):
        out_c = np.asarray(res.results[c]["out_t"])  # (NV, BATCH) bf16
        out_full[:, c * NV : (c + 1) * NV] = out_c.T.astype(np.float32)
    return out_full
